# revision 78
# baseline (speedup 1.0000x reference)
"""Joint bilateral filter (3x3, reflect pad) on 8 trn2 cores.

Sharding: 1024 output rows (2 batches x 512 H) split as 8 x 128 rows.
Host pre-pads H and W with reflect (radius 1), so each core gets a
halo-inclusive channel-major shard and computes its [128, C, 512]
output slab with no boundary handling on device.

Device layout: partition p = output row p of the shard. Free dim is
channel-major [C, W] so the per-pixel bilateral weight (one per W pos)
broadcasts across channels via a stride-0 AP, and dx shifts are free-dim
offsets. dy shifts are handled by loading 3 row-shifted copies of the
inputs (dy = 0,1,2 -> padded rows [dy, dy+128)).

Engine split (final), per column chunk:
- DVE:  guide-difference subs and 6 of the 8 non-center tap products in
  bf16 2x packed mode (k=7 dx=1 reads a host-prepared one-element-
  shifted copy for 4B alignment), reciprocal of den, final num*(1/den).
- Act:  square + exp for the 7 computed weight fields, the w5 shift
  copy (below), and the PSUM->SBUF copies of num.
- Pool: channel-sum of the squared guide distances, den accumulation,
  and the k=1 dx=1 tap product (Pool has no packed-alignment modes, so
  it reads the odd offset directly). The k=1 product is emitted at the
  top of each iteration so it lands before PE wants it (it is PE's
  last accumulate).
- PE:   sums the 8 tap products + the center src into PSUM fp32 via
  identity-weight matmuls (psum += I @ prod) - the adds cost the
  otherwise-idle tensor engine ~1.1us per tap instead of 1.5us of DVE,
  and the fp32 PSUM accumulation improves accuracy. A filler matmul
  between taps keeps the PE busy through prod waits: its DVFS reaches
  2.4GHz only after 3us of gap-free execution and any bubble resets
  it. num is split into two half-width PSUM tiles so the next chunk's
  PE work only waits on the matching half's PSUM->SBUF copy.
- tap symmetry: w5[p,x] = w3[p,x+1] exactly (opposite taps share their
  guide-difference field; the dy=1 pair needs no row shift), so tap
  5's weight pipeline is replaced by computing w3 one column wider and
  one tiny SBUF->SBUF shift-DMA. The row-shifted pairs (w6/w7/w8)
  cannot use this: the BIR verifier rejects cross-partition SBUF DMAs.
- weights(ci+1) is emitted before mac(ci) (software pipelining), so
  the weight pipeline always has a chunk of lead time; each chunk's
  finalize (reciprocal etc) is emitted one chunk late so the in-order
  DVE stream never stalls on den/num completion.
- src arrives as 4 column-chunks per row-shifted copy, pre-chunked in
  DRAM by the host so each slab DMA is one contiguous run per
  partition and chunk-0 compute starts ~4us in.
- center tap weight is exactly 1: PE accumulates src directly, +1 via
  den's memset.
"""

import sys

sys.path.insert(0, "/opt/trn_rl_repo")

import ml_dtypes
import numpy as np

BF16 = ml_dtypes.bfloat16

B, H, W = 2, 512, 512
CS, CI = 21, 3
N_CORES = 8
ROWS = (B * H) // N_CORES  # 128 output rows per core
WP = W + 2  # padded width
CHUNK = 128  # output cols per compute chunk
N_CHUNKS = W // CHUNK
SLAB = CHUNK + 2  # chunk + dx halo

SIGMA_BILATERAL = 0.25
INV2SIG2 = 1.0 / (2.0 * SIGMA_BILATERAL**2)  # 8.0

NONCENTER = [0, 1, 2, 3, 5, 6, 7, 8]

_CACHE = {}


def _build():
    from concourse.bacc import Bacc
    from concourse.tile import TileContext
    import concourse.mybir as mybir

    fp32 = mybir.dt.float32
    bf16 = mybir.dt.bfloat16
    Alu = mybir.AluOpType
    Act = mybir.ActivationFunctionType

    nc = Bacc("TRN2", target_bir_lowering=False, debug=False, num_devices=N_CORES)
    # se: x = w0 + [0, 130); so: dy in (0, 2) rows, x = w0 + 1 + [0, 130)
    se_d = nc.dram_tensor(
        "src_e", [N_CHUNKS, ROWS + 2, CS, SLAB], bf16, kind="ExternalInput"
    )
    so_d = nc.dram_tensor(
        "src_o2", [N_CHUNKS, ROWS, CS, SLAB], bf16, kind="ExternalInput"
    )
    im_d = nc.dram_tensor("im", [ROWS + 2, CI, WP], bf16, kind="ExternalInput")
    id_d = nc.dram_tensor("ident", [ROWS, ROWS], bf16, kind="ExternalInput")
    out_d = nc.dram_tensor("out", [ROWS, CS, W], bf16, kind="ExternalOutput")

    # num PSUM half-windows: channel ranges of <= 8 channels
    # (8*64 = 512 fp32, one 2KB bank per window)
    WIN2 = [(c0, min(CS, c0 + 8)) for c0 in range(0, CS, 8)]

    with TileContext(nc) as tc:
        with tc.tile_pool(name="p", bufs=1) as pool, tc.tile_pool(
            name="ps", bufs=1, space="PSUM"
        ) as ppool:
            bias_t = {}
            for v in (0.0, -0.5, -1.0):
                bt = pool.tile([ROWS, 1], fp32, tag=f"b{v}")
                nc.gpsimd.memset(bt[:], v)
                bias_t[v] = bt

            # im + identity first (guide center i_t[1] gates everything),
            # then chunk-0 src slabs, then the rest; the SP DMA queue
            # serializes in this order.
            i_t = [None] * 3
            for dy in (1, 0):
                it = pool.tile([ROWS, CI, WP], bf16, tag=f"i{dy}")
                nc.sync.dma_start(
                    it[:].rearrange("p c w -> p (c w)"),
                    im_d[dy : dy + ROWS].rearrange("p c w -> p (c w)"),
                )
                i_t[dy] = it
            ident = pool.tile([ROWS, ROWS], bf16, tag="ident")
            nc.sync.dma_start(ident[:], id_d[:])

            s_e = [[None] * 3 for _ in range(N_CHUNKS)]
            s_o = [None] * N_CHUNKS  # dy = 2 only (k=7 on DVE needs align)

            def issue_slab(ci):
                for dy in range(3):
                    se = pool.tile([ROWS, CS, SLAB], bf16, tag=f"se{ci}_{dy}")
                    nc.sync.dma_start(
                        se[:].rearrange("p c w -> p (c w)"),
                        se_d[ci, dy : dy + ROWS].rearrange("p c w -> p (c w)"),
                    )
                    s_e[ci][dy] = se

            def issue_so(ci):
                # odd-shifted copy for the k=7 dx=1 tap
                so = pool.tile([ROWS, CS, SLAB], bf16, tag=f"so{ci}")
                nc.sync.dma_start(
                    so[:].rearrange("p c w -> p (c w)"),
                    so_d[ci].rearrange("p c w -> p (c w)"),
                )
                s_o[ci] = so

            it = pool.tile([ROWS, CI, WP], bf16, tag="i2")
            nc.sync.dma_start(
                it[:].rearrange("p c w -> p (c w)"),
                im_d[2 : 2 + ROWS].rearrange("p c w -> p (c w)"),
            )
            i_t[2] = it
            for ci in range(N_CHUNKS):
                issue_slab(ci)
                issue_so(ci)

            # Each consuming engine observes every input DMA once (tiny
            # absorber ops) so real consumers don't pile up sync waits.
            # DVE consumes src, Pool consumes im.
            dummV = pool.tile([1, 1, 1], bf16, tag="dummV")

            def absorb_src_slab(ci):
                for t in s_e[ci] + [s_o[ci]]:
                    nc.vector.tensor_scalar(
                        dummV[:], t[0:1, 0:1, 0:1], 0.0, None, Alu.add
                    )

            dummP = pool.tile([1, 1, 1], bf16, tag="dummP")
            for t in i_t:
                nc.gpsimd.tensor_scalar(dummP[:], t[0:1, 0:1, 0:1], 0.0, None, Alu.add)

            # --- software-pipelined chunk loop ---------------------------
            # weights(ci) runs one chunk ahead of mac(ci): the DVE emits
            # subs(ci+1) before mults(ci), so the Pool/Act weight pipeline
            # always has a full chunk of lead time and never starves the
            # MAC engines.
            wk_all = [None] * N_CHUNKS
            den_all = [None] * N_CHUNKS
            pp1_all = [None] * N_CHUNKS

            def pp1(ci):
                # k=1 tap product on Pool (no packed-alignment modes, reads
                # the odd offset directly); emitted at the TOP of the
                # iteration so Pool delivers it before PE needs it
                nc.gpsimd.tensor_scalar(
                    dummP[:], s_e[ci][0][0:1, 0:1, 0:1], 0.0, None, Alu.add
                )
                wk_b = wk_all[ci][1][:, 0:CHUNK].rearrange(
                    "p (x w) -> p x w", x=1
                ).broadcast_to([ROWS, CS, CHUNK])
                pt = pool.tile([ROWS, CS, CHUNK], bf16, tag="prod1")
                nc.gpsimd.tensor_tensor(
                    pt[:], s_e[ci][0][:, :, 1 : 1 + CHUNK], wk_b, Alu.mult
                )
                pp1_all[ci] = pt

            def weights(ci):
                w0 = ci * CHUNK
                den = pool.tile([ROWS, CHUNK], fp32, tag=f"den{ci}")
                nc.gpsimd.memset(den[:], 1.0)
                den_all[ci] = den
                wk = {}
                # k=5 is never computed: w5[p,x] = w3[p,x+1] (opposite taps
                # share their guide-difference field and the dy=1 pair needs
                # no row shift), so k=3 is computed one column wider and k=5
                # is a one-column shift-copy of it.
                for k in (3, 0, 1, 2, 6, 7, 8):
                    dy, dx = k // 3, k % 3
                    cw = CHUNK + 1 if k == 3 else CHUNK
                    ic = i_t[1][:, :, w0 + 1 : w0 + 1 + cw]
                    lnw1 = -0.5 * ((dx - 1) ** 2 + (dy - 1) ** 2)
                    dtag = f"d3w{ci % 3}" if k == 3 else f"d{ci % 3}_{k % 2}"
                    d = pool.tile([ROWS, CI, cw], bf16, tag=dtag)
                    nc.vector.tensor_tensor(
                        d[:], i_t[dy][:, :, w0 + dx : w0 + dx + cw], ic,
                        Alu.subtract,
                    )
                    d2tag = f"d23w{ci % 3}" if k == 3 else f"d2{ci % 3}_{k % 2}"
                    d2 = pool.tile([ROWS, CI, cw], fp32, tag=d2tag)
                    nc.scalar.square(d2[:], d[:])
                    # sum over the 3 guide channels (Pool has no free-dim
                    # reduce, so two explicit adds)
                    wrtag = f"wr3w{ci % 3}" if k == 3 else f"wr{ci % 3}_{k % 2}"
                    wr = pool.tile([ROWS, cw], fp32, tag=wrtag)
                    nc.gpsimd.tensor_tensor(
                        wr[:], d2[:, 0, :], d2[:, 1, :], Alu.add
                    )
                    nc.gpsimd.tensor_tensor(wr[:], d2[:, 2, :], wr[:], Alu.add)
                    wt = pool.tile([ROWS, cw], bf16, tag=f"wk{ci}_{k}")
                    nc.scalar.activation(
                        wt[:], wr[:], Act.Exp, bias=bias_t[lnw1][:], scale=-INV2SIG2
                    )
                    wk[k] = wt
                # w5 <- w3 shifted: issued on the Act DGE queue right after
                # exp(k=3), so same-engine program order replaces a sem wait
                w5 = pool.tile([ROWS, CHUNK], bf16, tag=f"wk{ci}_5")
                nc.scalar.dma_start(w5[:], wk[3][:, 1 : 1 + CHUNK])
                wk[5] = w5
                wk_all[ci] = wk

            def mac(ci):
                w0 = ci * CHUNK
                wk = wk_all[ci]
                # tap products: k=1 on Pool (it has slack), rest on DVE in
                # bf16 2x packed mode; dx=1 taps last (their shifted copies
                # arrive after the main slab).
                absorb_src_slab(ci)
                nc.vector.tensor_scalar(
                    dummV[:], wk[5][0:1, 0:1], 0.0, None, Alu.add
                )
                nc.gpsimd.tensor_scalar(
                    dummP[:], wk[5][0:1, 0:1], 0.0, None, Alu.add
                )
                den = den_all[ci]
                for k in (0, 1, 2, 3, 6, 7, 8, 5):
                    nc.gpsimd.tensor_tensor(
                        den[:], wk[k][:, 0:CHUNK], den[:], Alu.add
                    )
                prods = {}
                prods[1] = pp1_all[ci]
                for k in (0, 2, 3, 6, 8, 5, 7):
                    dy, dx = k // 3, k % 3
                    wk_b = wk[k][:, 0:CHUNK].rearrange(
                        "p (x w) -> p x w", x=1
                    ).broadcast_to([ROWS, CS, CHUNK])
                    if k == 7:
                        sk = s_o[ci][:, :, 0:CHUNK]
                    else:
                        sk = s_e[ci][dy][:, :, dx : dx + CHUNK]
                    pt = pool.tile([ROWS, CS, CHUNK], bf16, tag=f"prod{k}")
                    nc.vector.tensor_tensor(pt[:], sk, wk_b, Alu.mult)
                    prods[k] = pt

                # PE: num = sum of products + center src (read directly, PE
                # has no alignment modes), accumulated in PSUM fp32 via
                # identity matmuls. Center first (always ready); a filler
                # matmul between taps keeps the PE busy through prod waits
                # (PE DVFS reaches 2.4GHz only after 3us gap-free). num is
                # split into two half-width PSUM tiles so the next chunk's
                # PE work only waits on the matching half's PSUM->SBUF copy.
                half = CHUNK // 2
                numps = []
                for h in range(2):
                    np_h = ppool.tile([ROWS, CS, half], fp32, tag=f"nump{h}")
                    numps.append(np_h)
                fill = ppool.tile([ROWS, 512], fp32, tag="fill")

                def filler(n):
                    for _ in range(n):
                        nc.tensor.matmul(
                            fill[:], ident[:],
                            s_e[ci][0][:, 0:4, 0:CHUNK],
                            start=True, stop=True,
                        )

                acc_srcs = [s_e[ci][1][:, :, 1 : 1 + CHUNK]]
                acc_srcs += [prods[k][:] for k in (0, 2, 3, 5, 6, 7, 8, 1)]
                n_acc = len(acc_srcs)
                for t, ap in enumerate(acc_srcs):
                    for h in range(2):
                        for a, b in WIN2:
                            nc.tensor.matmul(
                                numps[h][:, a:b, :],
                                ident[:],
                                ap[:, a:b, h * half : (h + 1) * half],
                                start=(t == 0), stop=(t == n_acc - 1),
                            )
                    if t < n_acc - 1:
                        filler(1)

                # Act: PSUM -> SBUF (bf16) per-half copies of num
                numb = pool.tile([ROWS, CS, CHUNK], bf16, tag="numb", bufs=2)
                for h in range(2):
                    nc.scalar.copy(
                        numb[:, :, h * half : (h + 1) * half], numps[h][:]
                    )

                def finalize():
                    den = den_all[ci]
                    rd = pool.tile([ROWS, CHUNK], fp32, tag=f"rd{ci}")
                    nc.vector.reciprocal(rd[:], den[:])
                    rdb = pool.tile([ROWS, CHUNK], bf16, tag=f"rdb{ci}")
                    nc.vector.tensor_scalar(rdb[:], rd[:], 0.0, None, Alu.add)
                    outt = pool.tile([ROWS, CS, CHUNK], bf16, tag="outt", bufs=2)
                    # last chunk: two half-width pieces so the final output
                    # DMA overlaps the second outmult instead of trailing it
                    parts = (
                        [(0, CHUNK // 2), (CHUNK // 2, CHUNK)]
                        if ci == N_CHUNKS - 1
                        else [(0, CHUNK)]
                    )
                    for a, b in parts:
                        rdb_b = rdb[:, a:b].rearrange(
                            "p (x w) -> p x w", x=1
                        ).broadcast_to([ROWS, CS, b - a])
                        nc.vector.tensor_tensor(
                            outt[:, :, a:b], numb[:, :, a:b], rdb_b, Alu.mult
                        )
                        nc.sync.dma_start(
                            out_d[:, :, w0 + a : w0 + b], outt[:, :, a:b]
                        )

                return finalize

            pending_finalize = None
            weights(0)
            for ci in range(N_CHUNKS):
                pp1(ci)
                if ci + 1 < N_CHUNKS:
                    weights(ci + 1)
                fin = mac(ci)
                if pending_finalize is not None:
                    pending_finalize()
                pending_finalize = fin
            pending_finalize()
    nc.compile()
    return nc


def _get_nc():
    if "nc" not in _CACHE:
        _CACHE["nc"] = _build()
    return _CACHE["nc"]


def _shard_inputs(src, im):
    srcp = np.pad(src, ((0, 0), (1, 1), (1, 1), (0, 0)), mode="reflect")
    imp = np.pad(im, ((0, 0), (1, 1), (1, 1), (0, 0)), mode="reflect")
    # channel-major: [B, Hp, C, Wp], bf16; pad 2 junk cols so the odd-shift
    # slab slices below stay in range
    srcp = np.transpose(srcp, (0, 1, 3, 2)).astype(BF16)
    srcp = np.pad(srcp, ((0, 0), (0, 0), (0, 0), (0, 2)))
    imp = np.ascontiguousarray(np.transpose(imp, (0, 1, 3, 2))).astype(BF16)
    ident = np.eye(ROWS, dtype=BF16)
    in_maps = []
    for core in range(N_CORES):
        b, r0 = core // 4, (core % 4) * ROWS
        sl = srcp[b, r0 : r0 + ROWS + 2]  # [130, 21, 516]
        se = np.stack(
            [sl[:, :, ci * CHUNK : ci * CHUNK + SLAB] for ci in range(N_CHUNKS)]
        )
        so = np.stack(
            [
                sl[2 : 2 + ROWS, :, ci * CHUNK + 1 : ci * CHUNK + 1 + SLAB]
                for ci in range(N_CHUNKS)
            ]
        )
        in_maps.append(
            {
                "src_e": np.ascontiguousarray(se),
                "src_o2": np.ascontiguousarray(so),
                "im": np.ascontiguousarray(imp[b, r0 : r0 + ROWS + 2]),
                "ident": ident,
            }
        )
    return in_maps


def kernel(src, im, _trace=False, _tmpdir=None):
    from concourse import bass_utils

    src = np.asarray(src, dtype=np.float32)
    im = np.asarray(im, dtype=np.float32)
    nc = _get_nc()
    in_maps = _shard_inputs(src, im)
    res = bass_utils.run_bass_kernel_spmd(
        nc, in_maps, core_ids=list(range(N_CORES)), trace=_trace, tmpdir=_tmpdir
    )
    out = np.empty((B, H, W, CS), dtype=np.float32)
    for core in range(N_CORES):
        b, r0 = core // 4, (core % 4) * ROWS
        o = res.results[core]["out"]  # [128, 21, 512] bf16
        out[b, r0 : r0 + ROWS] = np.transpose(o, (0, 2, 1)).astype(np.float32)
    _CACHE["last_results"] = res
    return out
